# revision 40
# baseline (speedup 1.0000x reference)
"""Trainium2 Bass kernel for nn_MultiHeadAttention_90924457656943.

Strategy (8 NeuronCores, SPMD), v2:
  - Row-shard the 2048 (b,s) token rows; each core computes q/k/v for its
    256 rows with FULL weights in bf16 (double projections in transposed
    space).  q+k ship in ONE AllToAll (transposed: head-dim on partitions);
    v ships natural in a second AllToAll.
  - After the A2As core j owns effective heads [8j, 8j+8) for all 2048
    rows.  Attention runs 4 heads at a time: scores are 4-way row-packed
    on the PE (tile_position rows 0/32/64/96, K=16 each, concurrent),
    exp'd straight out of PSUM with the 1/sqrt(16) scale folded in, and
    attn@v is 4-way column-packed (tile_position cols 0/32/64/96, M=17
    with a ones-column appended to v so the softmax denominators fall out
    of the same matmul).
  - exp is split across engines per key-chunk: ScalarE uses the real
    activation; DVE/Pool chunks use a Schraudolph bit-trick
    (i16 = x*C1 + C2 bitcast as bf16) in a single tensor_scalar op.
  - Softmax denominators never leave the chip: strided SBUF gathers
    collect them, DVE reciprocates, and tiny 0/1-stationary matmuls
    broadcast the reciprocals into the normalization multiplier tiles.
  - The module's quirky head-merge is a fixed permutation computed by
    constant 0/1 matmuls (4-way row-packed); normalized output ships
    through a third AllToAll and the final projection runs transposed.
"""

import os
import numpy as np
import ml_dtypes

STAGE = int(os.environ.get("K_STAGE", "3"))
SUB = int(os.environ.get("K_SUB", "9"))

import concourse.bass as bass
import concourse.tile as tile
from concourse import bacc, mybir
from concourse.bass_utils import run_bass_kernel_spmd

F32 = mybir.dt.float32
F32R = mybir.dt.float32r
BF16 = mybir.dt.bfloat16
I16 = mybir.dt.int16
AF = mybir.ActivationFunctionType
ALU = mybir.AluOpType

B, S, F = 2, 1024, 1024
H = 16          # head dim
C = 64          # effective heads
NCORES = 8
ROWS = (B * S) // NCORES          # 256 token rows per core
KC = F // 128                     # 8 contraction chunks

# exp engine per key-chunk: 'a' = ScalarE activation (exact),
# 'v' = DVE bit-trick, 'p' = Pool bit-trick
DVE_EXP = {(1, 1), (2, 1), (3, 1), (5, 1), (6, 1)}
# Schraudolph constants for exp(0.25*s) via int16 bitcast to bf16
EXP_C1 = 0.25 * 128 * 1.4426950408889634
EXP_C2 = 16250.65


def _perm_mats() -> np.ndarray:
    """32 constant matrices P[v,r,u]: rows 32v+h -> cols 64u+16r+h."""
    P = np.zeros((4, 4, 2, 128, 128), dtype=np.float32)
    for v in range(4):
        for r in range(4):
            for u in range(2):
                for h in range(H):
                    P[v, r, u, 32 * v + h, 64 * u + 16 * r + h] = 1.0
    return P.reshape(32, 128, 128)


WNAMES = ("wq_w", "vq_w", "wk_w", "vk_w", "wv_w", "wo_w")
BNAMES = ("wq_b", "vq_b", "wk_b", "vk_b", "wv_b", "vv_b", "wo_b")


def _build():
    nc = bacc.Bacc("TRN2", target_bir_lowering=False, debug=False,
                   num_devices=NCORES)

    xT = nc.dram_tensor("xT", [F, ROWS], BF16, kind="ExternalInput")
    # layout [m, p, k, j]: W[128k+p, 128m+j] -> contiguous (p, k*128+j) per m
    W = {n: nc.dram_tensor(n, [KC, 128, KC, 128], BF16, kind="ExternalInput")
         for n in WNAMES}
    vv_plain = nc.dram_tensor("vv_plain", [F, F], BF16, kind="ExternalInput")
    Bv = {n: nc.dram_tensor(n, [128, KC], F32, kind="ExternalInput")
          for n in BNAMES}
    b2vf = nc.dram_tensor("b2vf", [F], F32, kind="ExternalInput")
    perm = nc.dram_tensor("perm", [128, 32, 128], BF16, kind="ExternalInput")
    outT = nc.dram_tensor("outT", [F, ROWS], F32, kind="ExternalOutput")

    # internal DRAM A2A bounce buffers (all bf16)
    a2aq_in = nc.dram_tensor("a2aq_in", [NCORES, 128, ROWS], BF16)
    a2aq_out = nc.dram_tensor("a2aq_out", [NCORES, 128, ROWS], BF16)
    a2ak_in = nc.dram_tensor("a2ak_in", [NCORES, 128, ROWS], BF16)
    a2ak_out = nc.dram_tensor("a2ak_out", [NCORES, 128, ROWS], BF16)
    a2av_in = nc.dram_tensor("a2av_in", [NCORES, ROWS, 128], BF16)
    a2av_out = nc.dram_tensor("a2av_out", [NCORES, ROWS, 128], BF16)
    a2ao_in = nc.dram_tensor("a2ao_in", [NCORES, 128, ROWS], BF16)
    a2ao_out = nc.dram_tensor("a2ao_out", [NCORES, 128, ROWS], BF16)
    dn_dram = nc.dram_tensor("dn_bounce", [2, 2, 4, S], F32)
    rec_dram = nc.dram_tensor("rec_bounce", [2, 8, S], F32)

    RG = [list(range(NCORES))]

    def a2a(dst, src):
        nc.gpsimd.collective_compute(
            "AllToAll", ALU.bypass,
            ins=[src[:]], outs=[dst[:]], replica_groups=RG)

    from contextlib import ExitStack
    with tile.TileContext(nc) as tc, ExitStack() as _stk:
        # ---------- persistent pools ----------
        const_pool = _stk.enter_context(tc.tile_pool(name="const", bufs=1))
        bcol = {}
        for n in BNAMES:
            t = const_pool.tile([128, KC], F32, tag=f"b_{n}")
            nc.sync.dma_start(out=t[:], in_=Bv[n].ap())
            bcol[n] = t
        b2v_bc = const_pool.tile([128, F], F32, tag="b2v_bc")
        nc.sync.dma_start(out=b2v_bc[:], in_=b2vf.ap().partition_broadcast(128))
        # preload the Exp activation table before it's on the critical path
        actwarm = const_pool.tile([1, 2], F32, tag="actwarm")
        nc.scalar.activation(actwarm[0:1, 0:1], bcol["wq_b"][0:1, 0:1],
                             AF.Exp)

        # ---------- phase 1: projections ----------
        with tc.tile_pool(name="ppsum", bufs=4, space="PSUM") as ppsum, \
             tc.tile_pool(name="vpsum", bufs=2, space="PSUM") as vpsum, \
             tc.tile_pool(name="wpool", bufs=6) as wpool, \
             tc.tile_pool(name="wvw", bufs=1) as wvwpool, \
             tc.tile_pool(name="wvpool", bufs=10) as wvpool, \
             tc.tile_pool(name="ypool", bufs=2) as ypool, \
             tc.tile_pool(name="stage", bufs=6) as stage:

            wq_engs = [nc.sync, nc.scalar, nc.sync, nc.scalar]

            xt = []
            for k in range(KC):
                t = ypool.tile([128, ROWS], BF16, tag=f"xt{k}")
                nc.sync.dma_start(out=t[:], in_=xT[128 * k:128 * (k + 1), :])
                xt.append(t)

            def projT(wname, bname, rhs_tiles, ytag, out_dt=BF16,
                      wts=None):
                out_tiles = []
                for m in range(KC):
                    ps = ppsum.tile([128, ROWS], F32, tag="pp")
                    if wts is not None:
                        wt = wts[m]
                    else:
                        wt = wpool.tile([128, KC * 128], BF16, tag="w")
                        wq_engs[m % 4].dma_start(
                            out=wt[:].rearrange("p (k f) -> p k f", k=KC),
                            in_=W[wname][m])
                    for k in range(KC):
                        nc.tensor.matmul(ps[:], wt[:, 128 * k:128 * (k + 1)],
                                         rhs_tiles[k][:],
                                         start=(k == 0), stop=(k == KC - 1))
                    ot = ypool.tile([128, ROWS], out_dt, tag=f"{ytag}{m}")
                    if m % 2:
                        nc.vector.tensor_scalar_add(
                            ot[:], ps[:], bcol[bname][:, m:m + 1])
                    else:
                        nc.scalar.activation(
                            ot[:], ps[:], AF.Identity,
                            bias=bcol[bname][:, m:m + 1])
                    out_tiles.append(ot)
                return out_tiles

            # prefetch the v first-stage weights so the v projection can
            # fill the q/k A2A window
            wv_tiles = []
            for m in range(KC):
                wt = wvwpool.tile([128, KC * 128], BF16, tag=f"wvw{m}")
                wq_engs[m % 4].dma_start(
                    out=wt[:].rearrange("p (k f) -> p k f", k=KC),
                    in_=W["wv_w"][m])
                wv_tiles.append(wt)

            # q and k projections, then their A2As back-to-back
            y1q = projT("wq_w", "wq_b", xt, "y1")
            y2q = projT("vq_w", "vq_b", y1q, "y2")
            for m in range(KC):
                wq_engs[m % 4].dma_start(out=a2aq_in[m], in_=y2q[m][:])
            y1k = projT("wk_w", "wk_b", xt, "y1")
            y2k = projT("vk_w", "vk_b", y1k, "y2")
            for m in range(KC):
                wq_engs[m % 4].dma_start(out=a2ak_in[m], in_=y2k[m][:])
            a2a(a2aq_out, a2aq_in)
            a2a(a2ak_out, a2ak_in)

            # v: first projection transposed, second natural
            y1v = projT("wv_w", "wv_b", xt, "y1", wts=wv_tiles)
            for mb in range(ROWS // 128):
                for n2 in range(F // 512):
                    ps = vpsum.tile([128, 512], F32, tag="vp")
                    for k in range(KC):
                        wt = wvpool.tile([128, 512], BF16, tag="wv")
                        wq_engs[k % 4].dma_start(
                            out=wt[:],
                            in_=vv_plain[128 * k:128 * (k + 1),
                                         512 * n2:512 * (n2 + 1)])
                        nc.tensor.matmul(
                            ps[:], y1v[k][:, 128 * mb:128 * (mb + 1)], wt[:],
                            start=(k == 0), stop=(k == KC - 1))
                    ot = stage.tile([128, 512], BF16, tag="vout")
                    nc.vector.tensor_add(ot[:], ps[:],
                                         b2v_bc[:, 512 * n2:512 * (n2 + 1)])
                    for mm in range(4):
                        wq_engs[mm % 4].dma_start(
                            out=a2av_in[4 * n2 + mm,
                                        128 * mb:128 * (mb + 1), :],
                            in_=ot[:, 128 * mm:128 * (mm + 1)])
            a2a(a2av_out, a2av_in)
        if STAGE == 0:
            with tc.tile_pool(name="dbg", bufs=1) as dbg:
                t = dbg.tile([128, ROWS], BF16, tag="dbg0")
                for n in range(KC):
                    nc.gpsimd.dma_start(out=t[:, 0:128],
                                        in_=a2av_out[n, 0:128, :])
                    nc.gpsimd.dma_start(out=outT[128 * n:128 * (n + 1), :],
                                        in_=t[:])
            nc.finalize()
            return nc

        # ---------- phase 2: attention ----------
        wopool = _stk.enter_context(tc.tile_pool(name="wo", bufs=1))
        wo_tiles = []
        for n in range(KC):
            wt = wopool.tile([128, KC * 128], BF16, tag=f"wo{n}")
            nc.sync.dma_start(
                out=wt[:].rearrange("p (k f) -> p k f", k=KC),
                in_=W["wo_w"][n])
            wo_tiles.append(wt)
        popool = _stk.enter_context(tc.tile_pool(name="po", bufs=1))
        perm_sb = popool.tile([128, 32 * 128], BF16, tag="perm")
        nc.sync.dma_start(
            out=perm_sb[:].rearrange("p (n f) -> p n f", n=32),
            in_=perm.ap())

        def psl(i):
            return perm_sb[:, 128 * i:128 * (i + 1)]

        # (b2, g2) -> unnormalized o^T tile: head cl=4*g2+j at rows 32j,
        # denominators at rows 32j+16, columns = b2's 1024 tokens
        onpool = _stk.enter_context(tc.tile_pool(name="on", bufs=1))
        on_tiles = {}
        for _b2 in range(2):
            for _g2 in range(2):
                on_t = onpool.tile([128, S], BF16, tag=f"on{2 * _b2 + _g2}")
                on_tiles[(_b2, _g2)] = on_t
        dnpool = _stk.enter_context(tc.tile_pool(name="dn", bufs=1))
        mtpool = _stk.enter_context(tc.tile_pool(name="mt", bufs=1))

        blocks = [(0, 0), (0, 1), (1, 0), (1, 1)]

        with tc.tile_pool(name="qk", bufs=3) as qkpool, \
             tc.tile_pool(name="vt", bufs=3) as vtpool, \
             tc.tile_pool(name="ex", bufs=3) as expool, \
             tc.tile_pool(name="scp", bufs=3, space="PSUM") as scpsum, \
             tc.tile_pool(name="avp", bufs=2, space="PSUM") as avpsum, \
             tc.tile_pool(name="pmp", bufs=2, space="PSUM") as pmpsum, \
             tc.tile_pool(name="pst", bufs=4) as pstage:

            def load_block(b2, g2):
                """Prefetch q/k/v tiles for one 4-head block."""
                qs = qkpool.tile([128, S], BF16, tag="qs")
                ks = qkpool.tile([128, S], BF16, tag="ks")
                for m in range(4):
                    cl = 4 * g2 + m
                    nc.sync.dma_start(
                        out=qs[32 * m:32 * m + 16, :].rearrange(
                            "p (i f) -> p i f", i=4),
                        in_=a2aq_out[4 * b2:4 * (b2 + 1),
                                     16 * cl:16 * cl + 16, :].transpose(
                                         [1, 0, 2]))
                    nc.sync.dma_start(
                        out=ks[32 * m:32 * m + 16, :].rearrange(
                            "p (i f) -> p i f", i=4),
                        in_=a2ak_out[4 * b2:4 * (b2 + 1),
                                     16 * cl:16 * cl + 16, :].transpose(
                                         [1, 0, 2]))
                vts = []
                for kc in range(KC):
                    vt = vtpool.tile([128, 4 * 17], BF16, tag=f"vones{kc}")
                    ci = 4 * b2 + kc // 2
                    half = kc % 2
                    nc.gpsimd.dma_start(
                        out=vt[:].rearrange("p (m f) -> p m f",
                                            f=17)[:, :, 0:16],
                        in_=a2av_out[ci, 128 * half:128 * (half + 1),
                                     64 * g2:64 * (g2 + 1)].rearrange(
                                         "p (m f) -> p m f", m=4))
                    nc.gpsimd.memset(vt[:, 16::17].bitcast(BF16), 1.0)
                    vts.append(vt)
                return qs, ks, vts

            def attn_block(b2, g2, state):
                qs, ks, vts = state
                on = on_tiles[(b2, g2)]
                for q2 in range(4):
                    av = avpsum.tile([128, 256], F32, tag="av")
                    pend = []
                    for kc in range(KC):
                        sc = scpsum.tile([128, 1024], F32, tag="sc")
                        for j in range(4):
                            nc.tensor.matmul(
                                sc[:, 256 * j:256 * (j + 1)],
                                ks[32 * j:32 * j + 16,
                                   128 * kc:128 * (kc + 1)],
                                qs[32 * j:32 * j + 16,
                                   256 * q2:256 * (q2 + 1)],
                                start=True, stop=True,
                                tile_position=(32 * j, 0),
                                skip_group_check=True)
                        ex = expool.tile([128, 1024], BF16, tag="ex")
                        eng = EXP_ENG[kc]
                        if eng == 'a':
                            nc.scalar.activation(ex[:], sc[:], AF.Exp,
                                                 scale=0.25)
                        else:
                            e = nc.vector if eng == 'v' else nc.gpsimd
                            e.tensor_scalar(
                                out=ex[:].bitcast(I16), in0=sc[:],
                                scalar1=EXP_C1, scalar2=EXP_C2,
                                op0=ALU.mult, op1=ALU.add)
                        pend.append((kc, ex))
                        if len(pend) > 1:
                            flush(pend.pop(0), av, vts, last=False)
                    flush(pend.pop(0), av, vts, last=True)
                    # unnormalized o + denominators -> SBUF (bf16)
                    nc.vector.tensor_copy(
                        on[:, 256 * q2:256 * (q2 + 1)], av[:])

            def flush(item, av, vts, last):
                kc, ex = item
                for j in range(4):
                    nc.tensor.matmul(
                        av[32 * j:32 * j + 17, :],
                        vts[kc][:, 17 * j:17 * (j + 1)],
                        ex[:, 256 * j:256 * (j + 1)],
                        start=(kc == 0), stop=(kc == KC - 1),
                        tile_position=(0, 32 * j),
                        skip_group_check=True)

            def perm_phase(b2, dn2, jp_of):
                """Denominator gather + reciprocal + mt build + permute +
                normalize + stage one batch-half's A2A input."""
                rec = dnpool.tile([4, 2 * S], F32, tag=f"rec{b2}")
                nc.vector.reciprocal(rec[:], dn2[:])
                nc.gpsimd.dma_start(out=rec_dram[b2], in_=rec[:])
                mts = {}
                for vp in range(2):
                    mt = mtpool.tile([128, 512], F32, tag=f"mt{b2}{vp}")
                    for vh in range(2):
                        v_ = 2 * vp + vh
                        for u in range(2):
                            cl = 4 * u + v_
                            for r in range(4):
                                nc.gpsimd.dma_start(
                                    out=mt[64 * u + 16 * r:
                                           64 * u + 16 * (r + 1),
                                           256 * vh:256 * (vh + 1)],
                                    in_=bass.AP(
                                        tensor=rec_dram,
                                        offset=(b2 * 4 + r) * 2 * S
                                        + 256 * cl,
                                        ap=[[0, 16], [1, 256]]))
                    mts[vp] = mt
                pps = {}
                for v_ in range(4):
                    if v_ % 2 == 0:
                        pp = pmpsum.tile([128, 512], F32, tag="pp")
                        pps[v_ // 2] = pp
                    ppsl = pps[v_ // 2][:, 256 * (v_ % 2):256 * (v_ % 2 + 1)]
                    nmm = 0
                    for u in range(2):
                        src = on_tiles[(b2, u)]
                        for r in range(4):
                            pi = (v_ * 4 + r) * 2 + u
                            nc.tensor.matmul(
                                ppsl,
                                psl(pi)[32 * v_:32 * v_ + 16, :],
                                src[32 * v_:32 * v_ + 16, r::4],
                                start=(nmm == 0), stop=(nmm == 7),
                                tile_position=(32 * v_, 0),
                                skip_group_check=True)
                            nmm += 1
                for v_ in range(4):
                    ot = pstage.tile([128, 256], BF16, tag="pout")
                    nc.vector.tensor_mul(
                        ot[:],
                        pps[v_ // 2][:, 256 * (v_ % 2):256 * (v_ % 2 + 1)],
                        mts[v_ // 2][:, 256 * (v_ % 2):256 * (v_ % 2 + 1)])
                    nc.sync.dma_start(out=a2ao_in[jp_of(v_)], in_=ot[:])

            def gather_dn(b2, g2):
                # on rows 16+32j hold denominators for head cl=4*g2+j in
                # natural token order -> bounce to DRAM
                nc.gpsimd.dma_start(out=dn_dram[b2, g2],
                                    in_=on_tiles[(b2, g2)][16::32, :])

            def load_dn(b2, dn2):
                # perm output column a covers token 4a+r, so read back as
                # dn2[r, 256*cl + a] = denom[cl, 4a+r]
                for g2 in range(2):
                    nc.gpsimd.dma_start(
                        out=dn2[:, 1024 * g2:1024 * (g2 + 1)].rearrange(
                            "r (j a) -> r j a", j=4),
                        in_=dn_dram[b2, g2].rearrange(
                            "j (a r) -> r j a", r=4))

            # main schedule: prefetch one block ahead; perm(b2) emitted
            # right after both of its blocks finish
            state = load_block(*blocks[0])
            for bi, (b2, g2) in enumerate(blocks):
                nxt = load_block(*blocks[bi + 1]) if bi + 1 < 4 else None
                if g2 == 0:
                    dn2 = dnpool.tile([4, 2 * S], F32, tag=f"dn{b2}")
                else:
                    dn2 = dn2_prev
                attn_block(b2, g2, state)
                if STAGE >= 2:
                    gather_dn(b2, g2)
                    if g2 == 1:
                        load_dn(b2, dn2)
                        perm_phase(b2, dn2,
                                   lambda v_: ((v_ >> 1) & 1) * 4
                                   + (v_ & 1) * 2 + b2)
                dn2_prev = dn2
                state = nxt

            if STAGE >= 2:
                a2a(a2ao_out, a2ao_in)
        if STAGE == 1:
            for n in range(KC):
                hb = on_tiles[(n % 2, (n // 2) % 2)]
                nc.gpsimd.dma_start(out=outT[128 * n:128 * (n + 1), :],
                                     in_=hb[:, 0:ROWS])
            nc.finalize()
            return nc
        if STAGE == 2:
            with tc.tile_pool(name="dbg2", bufs=1) as dbg2:
                t2 = dbg2.tile([128, ROWS], BF16, tag="dbg2")
                for n in range(KC):
                    nc.gpsimd.dma_start(out=t2[:], in_=a2ao_out[n])
                    nc.gpsimd.dma_start(out=outT[128 * n:128 * (n + 1), :],
                                        in_=t2[:])
            nc.finalize()
            return nc

        # ---------- phase 3: output projection ----------
        with tc.tile_pool(name="oo", bufs=1) as oopool, \
             tc.tile_pool(name="ops", bufs=2, space="PSUM") as opsum:
            rhs_o = []
            for k in range(KC):
                t = oopool.tile([128, ROWS], BF16, tag=f"ro{k}")
                (nc.sync if k % 2 else nc.gpsimd).dma_start(
                    out=t[:], in_=a2ao_out[k])
                rhs_o.append(t)
            for n in range(KC):
                ps = opsum.tile([128, ROWS], F32, tag="op")
                wt = wo_tiles[n]
                for k in range(KC):
                    nc.tensor.matmul(ps[:], wt[:, 128 * k:128 * (k + 1)],
                                     rhs_o[k][:],
                                     start=(k == 0), stop=(k == KC - 1))
                ot = oopool.tile([128, ROWS], F32, tag="fout")
                nc.vector.tensor_scalar_add(ot[:], ps[:],
                                            bcol["wo_b"][:, n:n + 1])
                nc.sync.dma_start(out=outT[128 * n:128 * (n + 1), :],
                                  in_=ot[:])

    nc.finalize()
    return nc


_NC_CACHE = None


def _get_nc():
    global _NC_CACHE
    if _NC_CACHE is None:
        _NC_CACHE = _build()
    return _NC_CACHE


def _bf16(a):
    return np.asarray(a, dtype=np.float32).astype(ml_dtypes.bfloat16)


def kernel(x, wq_w, wq_b, wk_w, wk_b, wv_w, wv_b,
           vq_w, vq_b, vk_w, vk_b, vv_w, vv_b, wo_w, wo_b,
           _trace=False):
    nc = _get_nc()

    ws = {"wq_w": wq_w, "vq_w": vq_w, "wk_w": wk_w, "vk_w": vk_w,
          "wv_w": wv_w, "wo_w": wo_w}
    bs = {"wq_b": wq_b, "vq_b": vq_b, "wk_b": wk_b, "vk_b": vk_b,
          "wv_b": wv_b, "vv_b": vv_b, "wo_b": wo_b}

    wchunks = {n: np.ascontiguousarray(
        _bf16(w).reshape(KC, 128, KC, 128).transpose(2, 1, 0, 3))
        for n, w in ws.items()}
    bmap = {n: np.ascontiguousarray(
        np.asarray(b, dtype=np.float32).reshape(KC, 128).T)
            for n, b in bs.items()}
    b2vf_host = np.ascontiguousarray(np.asarray(vv_b, dtype=np.float32))
    vv_plain_host = _bf16(vv_w)
    P = np.ascontiguousarray(_bf16(_perm_mats()).transpose(1, 0, 2))

    xf = np.asarray(x, dtype=np.float32).reshape(B * S, F)
    in_maps = []
    for j in range(NCORES):
        xTj = np.ascontiguousarray(_bf16(xf[ROWS * j:ROWS * (j + 1)]).T)
        m = {"xT": xTj, "perm": P, "vv_plain": vv_plain_host,
             "b2vf": b2vf_host}
        m.update(wchunks)
        m.update(bmap)
        in_maps.append(m)

    kw = {}
    if _trace:
        import sys
        import types
        if "antenv.axon_hooks" not in sys.modules:
            import antenv
            mod = types.ModuleType("antenv.axon_hooks")
            mod._hook = None
            def _set(h):
                mod._hook = h
            def _get():
                return mod._hook
            mod.set_axon_ntff_profile_hook = _set
            mod.get_axon_ntff_profile_hook = _get
            sys.modules["antenv.axon_hooks"] = mod
            antenv.axon_hooks = mod
            from trn_agent_boot.trn_boot import _ntff_profile_via_ctypes
            _set(_ntff_profile_via_ctypes("/opt/axon/libaxon_pjrt.so"))
        kw = dict(trace=True, trace_cores=list(range(NCORES)))
    res = run_bass_kernel_spmd(nc, in_maps, core_ids=list(range(NCORES)), **kw)

    out = np.empty((B * S, F), dtype=np.float32)
    for j in range(NCORES):
        out[ROWS * j:ROWS * (j + 1)] = res.results[j]["outT"].T
    if _trace:
        return out.reshape(B, S, F), res
    return out.reshape(B, S, F)
